# revision 20
# baseline (speedup 1.0000x reference)
"""Distributed Trainium2 Bass kernel for causal multi-head attention with RoPE.

Problem: B=2, T=2048, C=1024, H=16 heads, D=64. 8 NeuronCores.

Sharding (2x4 grid): core c handles batch b = c//4 and the 4 heads
g = c%4 -> heads [4g..4g+4). QKV projections + RoPE + causal attention run
fully locally per core in a "transposed" layout (qT/kT = [D_heads, T]) so
no on-chip transposes are ever needed:

  qT = Wq_slice.T @ x.T            (lhsT = Wq natural, rhs = x.T)
  scoresT[k,q] = kT.T-block @ qT   (softmax along PARTITION axis)
  outT = [v|1].T @ exp(scoresT)    (ones column yields softmax denominators)
  outW = Wo_cols.T @ attn_allT     (attn stays transposed through Wo)

v2 schedule: the QKV+RoPE projection work (phase A), the attention inner
loop (phase B) and the Wo projection (phase C) are software-pipelined into
one interleaved emission stream so the PE never starves while the scalar
engine churns through the softmax exps.  Normalization uses a gpsimd
partition_broadcast of the reciprocal denominators instead of a PE
ones-matmul.  Per-(qc,pair) AllGathers move the normalized attention;
DMA loads are ordered so compute starts ~2us into the kernel.
"""

import numpy as np
import ml_dtypes

import concourse.bacc as bacc
import concourse.mybir as mybir
import concourse.tile as tile
from concourse.bass_utils import run_bass_kernel_spmd

B, T, C, H, D = 2, 2048, 1024, 16, 64
NCORES = 8
HPC = 4              # heads per core
CPC = HPC * D        # channels per core (256)
NPAIR = 2            # head pairs per core
QC = 4               # q-chunks of 512
KB = T // 128        # k-blocks of 128
CCH = C // 128       # contraction chunks of 128
F32 = mybir.dt.float32
BF16 = mybir.dt.bfloat16
AF = mybir.ActivationFunctionType
RGROUPS = [[0, 1, 2, 3], [4, 5, 6, 7]]

_cache = {}


def _build_nc():
    nc = bacc.Bacc(None, target_bir_lowering=False, debug=False, num_devices=NCORES)

    xT = nc.declare_dram_parameter("xT", [C, T], BF16, isOutput=False)
    wq = nc.declare_dram_parameter("wq", [C, CPC], BF16, isOutput=False)
    wk = nc.declare_dram_parameter("wk", [C, CPC], BF16, isOutput=False)
    wv = nc.declare_dram_parameter("wv", [C, CPC], BF16, isOutput=False)
    wo = nc.declare_dram_parameter("wo", [C, CPC], BF16, isOutput=False)
    # Wo row-slice (this core's 256 attn channels x all 1024 out channels)
    # for the ReduceScatter-based final q-chunk projection
    wor = nc.declare_dram_parameter("wor", [CPC, C], BF16, isOutput=False)
    cosP = nc.declare_dram_parameter("cosP", [128, T], F32, isOutput=False)
    sinP = nc.declare_dram_parameter("sinP", [128, T], F32, isOutput=False)
    maskut = nc.declare_dram_parameter("maskut", [128, 256], BF16, isOutput=False)
    smat = nc.declare_dram_parameter("smat", [128, 128], BF16, isOutput=False)
    out = nc.declare_dram_parameter("out", [CPC, T], F32, isOutput=True)
    out3 = nc.declare_dram_parameter("out3", [CPC, 512], BF16, isOutput=True)

    with tile.TileContext(nc) as tc:
        with (
            tc.tile_pool(name="resident", bufs=1) as rp,
            tc.tile_pool(name="rope", bufs=2) as ropep,
            tc.tile_pool(name="expp", bufs=8) as expp,
            tc.tile_pool(name="outb", bufs=4) as outbp,
            tc.tile_pool(name="normp", bufs=2) as normp,
            tc.tile_pool(name="agsb", bufs=3) as agp,
            tc.tile_pool(name="osbp", bufs=2) as osbp,
            tc.tile_pool(name="psS", bufs=2, space="PSUM") as psS,
            tc.tile_pool(name="psAV", bufs=1, space="PSUM") as psAV,
            tc.tile_pool(name="psA", bufs=2, space="PSUM") as psA,
            tc.tile_pool(name="dram", bufs=1, space="DRAM") as dram,
        ):
            # ---------------- resident SBUF ----------------
            xn = [rp.tile([128, CCH * 512], BF16, name=f"xn{i}") for i in range(4)]
            wqbf = rp.tile([128, CCH * CPC], BF16, name="wqbf")
            wkbf = rp.tile([128, CCH * CPC], BF16, name="wkbf")
            wvbf = rp.tile([128, CCH * CPC], BF16, name="wvbf")
            wobf = rp.tile([128, CCH * CPC], BF16, name="wobf")
            cos_n = [rp.tile([128, 512], F32, name=f"cosn{i}") for i in range(4)]
            sin_n = [rp.tile([128, 512], F32, name=f"sinn{i}") for i in range(4)]
            mask_bf = rp.tile([128, 256], BF16, name="maskbf")
            smat_bf = rp.tile([128, 128], BF16, name="smatbf")
            qTn = [rp.tile([128, NPAIR * 512], BF16, name=f"qTn{i}") for i in range(4)]
            kTn = [rp.tile([128, NPAIR * 512], BF16, name=f"kTn{i}") for i in range(4)]
            # per nch: [v | 1] per head per local k-block (4 blocks of 128)
            vsbn = [rp.tile([128, HPC * 4 * 65], BF16, name=f"vsbn{i}") for i in range(4)]

            worbf = rp.tile([128, 2 * C], BF16, name="worbf")
            partial_sb = rp.tile([128, 8 * 512], BF16, name="partialsb")

            # ---------------- initial loads, priority-ordered ----------------
            # batched multi-chunk DMAs: one trigger covers several 128-row
            # chunks; the HWDGE queues (sync/scalar) round-robin 16 physical
            # queues so the batches still transfer in parallel
            def wload(eng, sb, w, c0, c1):
                eng.dma_start(
                    sb[:, c0 * CPC:c1 * CPC].rearrange("p (cc n) -> p cc n", cc=c1 - c0),
                    w[c0 * 128:c1 * 128, :].rearrange("(cc p) n -> p cc n", p=128))

            def xload(eng, nch, c0, c1):
                eng.dma_start(
                    xn[nch][:, c0 * 512:c1 * 512].rearrange("p (cc n) -> p cc n", cc=c1 - c0),
                    xT[c0 * 128:c1 * 128, nch * 512:(nch + 1) * 512]
                    .rearrange("(cc p) n -> p cc n", p=128))

            # critical path: wq, x(nch0), cos0/sin0, smat -> compute at ~2us
            nc.scalar.dma_start(smat_bf[:], smat[:])
            wload(nc.sync, wqbf, wq, 0, 4)
            wload(nc.scalar, wqbf, wq, 4, 8)
            xload(nc.sync, 0, 0, 4)
            xload(nc.scalar, 0, 4, 8)
            nc.scalar.dma_start(cos_n[0][:], cosP[:, 0:512])
            nc.sync.dma_start(sin_n[0][:], sinP[:, 0:512])
            for i in range(4):
                nc.gpsimd.memset(vsbn[i][:], 1.0)

            # stage 2: remaining loads in rough need-order
            wload(nc.sync, wkbf, wk, 0, 4)
            wload(nc.scalar, wkbf, wk, 4, 8)
            wload(nc.sync, wvbf, wv, 0, 4)
            wload(nc.scalar, wvbf, wv, 4, 8)
            xload(nc.sync, 1, 0, 4)
            xload(nc.scalar, 1, 4, 8)
            nc.sync.dma_start(cos_n[1][:], cosP[:, 512:1024])
            nc.scalar.dma_start(sin_n[1][:], sinP[:, 512:1024])
            nc.sync.dma_start(mask_bf[:], maskut[:])
            xload(nc.sync, 2, 0, 4)
            xload(nc.scalar, 2, 4, 8)
            nc.sync.dma_start(cos_n[2][:], cosP[:, 1024:1536])
            nc.scalar.dma_start(sin_n[2][:], sinP[:, 1024:1536])
            xload(nc.sync, 3, 0, 4)
            xload(nc.scalar, 3, 4, 8)
            nc.sync.dma_start(cos_n[3][:], cosP[:, 1536:2048])
            nc.scalar.dma_start(sin_n[3][:], sinP[:, 1536:2048])
            wload(nc.sync, wobf, wo, 0, 4)
            wload(nc.scalar, wobf, wo, 4, 8)
            nc.sync.dma_start(
                worbf[:].rearrange("p (pp n) -> p pp n", pp=2),
                wor[:].rearrange("(pp p) n -> p pp n", p=128))

            # ---------------- phase A emitters (QKV + RoPE per nch) ----------
            def emit_qk_group(nch, w_sb, t_sb, p):
                ps_t = psA.tile([128, 512], F32, tag="a", name=f"pst{nch}_{id(w_sb)%7}_{p}")
                for cc in range(CCH):
                    nc.tensor.matmul(
                        ps_t[:],
                        w_sb[:, cc * CPC + p * 128: cc * CPC + (p + 1) * 128],
                        xn[nch][:, cc * 512:(cc + 1) * 512],
                        start=(cc == 0), stop=(cc == CCH - 1),
                    )
                qub = ropep.tile([128, 512], BF16, tag="qub")
                nc.vector.tensor_copy(qub[:], ps_t[:])
                rot = psA.tile([128, 512], F32, tag="a", name=f"rot{nch}_{id(w_sb)%7}_{p}")
                nc.tensor.matmul(rot[:], smat_bf[:], qub[:], start=True, stop=True)
                t1 = ropep.tile([128, 512], F32, tag="t1")
                nc.vector.tensor_mul(t1[:], ps_t[:], cos_n[nch][:])
                t2 = ropep.tile([128, 512], F32, tag="t2")
                nc.vector.tensor_mul(t2[:], rot[:], sin_n[nch][:])
                nc.vector.tensor_add(t_sb[:, p * 512:(p + 1) * 512], t1[:], t2[:])

            def emit_v_group(nch, tl):
                ps_v = psA.tile([128, 512], F32, tag="a", name=f"psv{nch}_{tl}")
                for cc in range(CCH):
                    nc.tensor.matmul(
                        ps_v[:, 0:CPC],
                        xn[nch][:, cc * 512 + tl * 128: cc * 512 + tl * 128 + 128],
                        wvbf[:, cc * CPC:(cc + 1) * CPC],
                        start=(cc == 0), stop=(cc == CCH - 1),
                    )
                # one strided copy scatters all 4 heads: [128, h, 64]
                dstv = vsbn[nch][:].rearrange("p (h b c) -> p h b c", h=HPC, b=4)[:, :, tl, 0:64]
                srcv = ps_v[:, 0:CPC].rearrange("p (h c) -> p h c", h=HPC)
                nc.vector.tensor_copy(dstv, srcv)

            def gen_A(nch):
                for p in range(NPAIR):
                    emit_qk_group(nch, wqbf, qTn[nch], p)
                    yield
                for p in range(NPAIR):
                    emit_qk_group(nch, wkbf, kTn[nch], p)
                    yield
                for tl in range(4):
                    emit_v_group(nch, tl)
                    yield

            # ---------------- phase B emitters (attention) --------------------
            bandst = [dram.tile([CPC, 512], BF16, tag=f"bd{qc}", name=f"band{qc}")
                      for qc in range(QC - 1)]
            agout = [dram.tile([4 * CPC, 512], BF16, tag=f"ag{qc}", name=f"agout{qc}")
                     for qc in range(QC - 1)]
            partial_dram = dram.tile([C, 512], BF16, tag="pd", name="partiald")

            def emit_scores(qc, p, kb, es):
                nqs = max(qc * 512, kb * 128)
                noff = nqs - qc * 512
                n = 512 - noff
                nch = kb // 4
                kl = kb % 4
                ps_s = psS.tile([128, 1024], F32, tag="s", name=f"pss{qc}_{p}_{kb}")
                for i in range(2):
                    hs = slice(i * 64, (i + 1) * 64)
                    nc.tensor.matmul(
                        ps_s[:, i * 512: i * 512 + n],
                        kTn[nch][hs, p * 512 + kl * 128: p * 512 + kl * 128 + 128],
                        qTn[qc][hs, p * 512 + noff: p * 512 + 512],
                        start=True, stop=True,
                        tile_position=(i * 64, 0),
                    )
                e = expp.tile([128, 1024], BF16, tag="e", name=f"e{qc}_{p}_{kb}")
                if noff:
                    for i in range(2):
                        nc.scalar.activation(e[:, i * 512: i * 512 + n],
                                             ps_s[:, i * 512: i * 512 + n],
                                             AF.Exp, scale=0.125)
                else:
                    nc.scalar.activation(e[:], ps_s[:], AF.Exp, scale=0.125)
                if nqs == kb * 128:  # diagonal block: causal mask
                    ev = e[:].rearrange("p (b c) -> p b c", b=2)[:, :, 0:128]
                    mv = mask_bf[:].rearrange("p (b c) -> p b c", b=2)
                    nc.vector.tensor_mul(ev, ev, mv)
                es[kb] = e

            def emit_av(qc, p, kb, av, e):
                nqs = max(qc * 512, kb * 128)
                noff = nqs - qc * 512
                n = 512 - noff
                nch = kb // 4
                kl = kb % 4
                kmax = 4 * qc + 4
                for i in range(2):
                    h = 2 * p + i
                    vbase = h * 4 * 65 + kl * 65
                    nc.tensor.matmul(
                        av[:, i * 512 + noff: (i + 1) * 512],
                        vsbn[nch][:, vbase: vbase + 65],
                        e[:, i * 512: i * 512 + n],
                        start=(kb == 0), stop=(kb == kmax - 1),
                    )

            _obs = {}

            def emit_norm(qc, p, av):
                # HW quirks: reciprocal_approx_fast misreads PSUM, and
                # partition_broadcast mis-writes dst at base-partition 64 —
                # so stage the denominators to SBUF and broadcast full-height
                dcp = normp.tile([1, 1024], F32, tag="dcp", name=f"dcp{qc}_{p}")
                nc.vector.tensor_copy(dcp[:], av[64:65, :])
                rec = normp.tile([1, 1024], F32, tag="rec", name=f"rec{qc}_{p}")
                nc.vector.reciprocal_approx_fast(rec[:], dcp[:])
                bcs = [normp.tile([128, 512], F32, tag=f"bc{i}", name=f"bc{qc}_{p}_{i}")
                       for i in range(2)]
                nc.gpsimd.partition_broadcast(bcs[0][:], rec[:, 0:512])
                nc.gpsimd.partition_broadcast(bcs[1][:], rec[:, 512:1024])
                ob = outbp.tile([128, 512], BF16, tag="ob", name=f"ob{qc}_{p}")
                for i in range(2):
                    nc.vector.tensor_mul(ob[i * 64:(i + 1) * 64, :],
                                         av[0:64, i * 512:(i + 1) * 512],
                                         bcs[i][i * 64:(i + 1) * 64, :])
                _obs[(qc, p)] = ob
                if qc < 3:
                    nc.sync.dma_start(bandst[qc][p * 128:(p + 1) * 128, :], ob[:])
                    if p == NPAIR - 1:
                        nc.gpsimd.collective_compute(
                            "AllGather", mybir.AluOpType.bypass,
                            replica_groups=RGROUPS,
                            ins=[bandst[qc].opt()], outs=[agout[qc].opt()],
                        )

            def gen_B(qc):
                for p in range(NPAIR):
                    kmax = 4 * qc + 4
                    lag = 4 if kmax > 4 else 4
                    av = None
                    es = {}
                    for j in range(kmax + lag):
                        if j < kmax:
                            emit_scores(qc, p, j, es)
                        if j >= lag:
                            kb = j - lag
                            if kb == 0:
                                av = psAV.tile([65, 1024], F32, tag="av",
                                               name=f"av{qc}_{p}")
                            emit_av(qc, p, kb, av, es.pop(kb))
                        yield
                    emit_norm(qc, p, av)
                    yield

            # ---------------- phase C emitters (Wo per qc) --------------------
            _ag_sb = {}

            def gen_Wo(qc):
                # one batched reload of the AllGathered attention, then two
                # m-chunk groups.  m-groups are emitted contiguously: the psA
                # pool ring (shared with phase A's ps_t/rot tiles) must never
                # interleave an open accumulation group with another psA
                # allocation, or the PE FIFO deadlocks on recycle semaphores.
                t = agp.tile([128, CCH * 512], BF16, name=f"ag_{qc}", tag="ag")
                nc.sync.dma_start(
                    t[:].rearrange("p (cc n) -> p cc n", cc=CCH),
                    agout[qc][:].rearrange("(cc p) n -> p cc n", p=128))
                _ag_sb[qc] = t
                yield
                for mch in range(2):
                    ps_o = psA.tile([128, 512], F32, tag="a", name=f"pso{qc}_{mch}")
                    for cc in range(CCH):
                        nc.tensor.matmul(
                            ps_o[:],
                            wobf[:, cc * CPC + mch * 128: cc * CPC + (mch + 1) * 128],
                            _ag_sb[qc][:, cc * 512:(cc + 1) * 512],
                            start=(cc == 0), stop=(cc == CCH - 1),
                        )
                    osb = osbp.tile([128, 512], F32, tag="osb")
                    nc.scalar.copy(osb[:], ps_o[:])
                    nc.sync.dma_start(out[mch * 128:(mch + 1) * 128,
                                          qc * 512:(qc + 1) * 512], osb[:])
                    yield

            def emit_wo3_tail():
                # final q-chunk: local Wo partials over this core's 256
                # channels for ALL 1024 output channels, then a ReduceScatter
                # sums the 4 group members' partials and hands each its slice.
                ob0, ob1 = _obs[(3, 0)], _obs[(3, 1)]
                for mch in range(8):
                    ps_p = psA.tile([128, 512], F32, tag="a", name=f"psp{mch}")
                    for p, ob in ((0, ob0), (1, ob1)):
                        nc.tensor.matmul(
                            ps_p[:],
                            worbf[:, p * C + mch * 128: p * C + (mch + 1) * 128],
                            ob[:],
                            start=(p == 0), stop=(p == 1),
                        )
                    eng = nc.scalar if mch % 2 == 0 else nc.vector
                    if mch % 2 == 0:
                        eng.copy(partial_sb[:, mch * 512:(mch + 1) * 512], ps_p[:])
                    else:
                        eng.tensor_copy(partial_sb[:, mch * 512:(mch + 1) * 512], ps_p[:])
                    if mch % 4 == 3:
                        h = mch // 4
                        nc.sync.dma_start(
                            partial_dram[h * 512:(h + 1) * 512, :]
                            .rearrange("(m p) n -> p m n", p=128),
                            partial_sb[:, h * 2048:(h + 1) * 2048]
                            .rearrange("p (m n) -> p m n", m=4))
                rs_out = dram.tile([CPC, 512], BF16, tag="rso", name="rsout")
                nc.gpsimd.collective_compute(
                    "ReduceScatter", mybir.AluOpType.add,
                    replica_groups=RGROUPS,
                    ins=[partial_dram.opt()], outs=[rs_out.opt()],
                )
                nc.sync.dma_start(out3[:], rs_out[:])

            # ---------------- interleaved emission schedule -------------------
            def weave(streams):
                # streams: list of (generator, n_steps, phase_offset)
                seq = []
                for idx, (g, n, off) in enumerate(streams):
                    for k in range(n):
                        seq.append(((k + off) / n, idx))
                seq.sort(key=lambda x: x[0])
                for _, idx in seq:
                    next(streams[idx][0], None)

            def a_len(nch):
                return 2 * NPAIR + 4

            def b_len(qc):
                return NPAIR * (4 * qc + 4 + 4 + 1)

            # S0: projections for tokens [0:512]
            for _ in gen_A(0):
                pass
            # S1: A(1) + B(0)
            weave([(gen_A(1), a_len(1), 0.5), (gen_B(0), b_len(0), 0.5)])
            # S2: A(2) + B(1) + Wo(0)
            weave([(gen_A(2), a_len(2), 0.5), (gen_B(1), b_len(1), 0.5),
                   (gen_Wo(0), 3, 1.0)])
            # S3: A(3) + B(2) + Wo(1)
            weave([(gen_A(3), a_len(3), 0.5), (gen_B(2), b_len(2), 0.5),
                   (gen_Wo(1), 3, 1.0)])
            # S4: B(3) + Wo(2)
            weave([(gen_B(3), b_len(3), 0.5), (gen_Wo(2), 3, 0.8)])
            # tail: Wo(3) via local partials + ReduceScatter
            emit_wo3_tail()
    return nc


def _get_nc():
    if "nc" not in _cache:
        nc = _build_nc()
        nc.finalize()
        _cache["nc"] = nc
    return _cache["nc"]


def _host_tables(freqs_cos, freqs_sin):
    cosP = np.empty((128, T), np.float32)
    sinP = np.empty((128, T), np.float32)
    for r in range(128):
        i = (r % 64) // 2
        cosP[r] = freqs_cos[:, i]
        sinP[r] = freqs_sin[:, i]
    maskut = np.tile(np.triu(np.ones((128, 128), np.float32)), (1, 2))
    smat = np.zeros((128, 128), np.float32)
    for i in range(64):
        smat[2 * i + 1, 2 * i] = -1.0   # rot[2i] = -q[2i+1]
        smat[2 * i, 2 * i + 1] = 1.0    # rot[2i+1] = +q[2i]
    return cosP, sinP, maskut, smat


def _install_trace_hooks():
    import sys, types
    try:
        import antenv.axon_hooks  # noqa: F401
        return True
    except ImportError:
        pass
    try:
        from trn_agent_boot.trn_boot import _ntff_profile_via_ctypes
        mod = types.ModuleType("antenv.axon_hooks")
        mod._hook = _ntff_profile_via_ctypes("/opt/axon/libaxon_pjrt.so")
        mod.set_axon_ntff_profile_hook = lambda h: setattr(mod, "_hook", h)
        mod.get_axon_ntff_profile_hook = lambda: mod._hook
        sys.modules["antenv.axon_hooks"] = mod
        import antenv
        antenv.axon_hooks = mod
        import concourse.bass_utils as bu
        bu.upload_artifacts = lambda tmpdir: f"file://{tmpdir}"
        return True
    except Exception:
        return False


def _bf16(a):
    return np.ascontiguousarray(a).astype(ml_dtypes.bfloat16)


def kernel(x, freqs_cos, freqs_sin, Wq, Wk, Wv, Wo, _trace=False):
    x = np.asarray(x, np.float32)
    freqs_cos = np.asarray(freqs_cos, np.float32)
    freqs_sin = np.asarray(freqs_sin, np.float32)
    Wq, Wk, Wv, Wo = (np.asarray(w, np.float32) for w in (Wq, Wk, Wv, Wo))
    cosP, sinP, maskut, smat = _host_tables(freqs_cos, freqs_sin)

    in_maps = []
    for c in range(NCORES):
        b, g = c // 4, c % 4
        sl = slice(g * CPC, (g + 1) * CPC)
        in_maps.append({
            "xT": _bf16(x[b].T),
            "wq": _bf16(Wq[:, sl]),
            "wk": _bf16(Wk[:, sl]),
            "wv": _bf16(Wv[:, sl]),
            "wo": _bf16(Wo[:, sl]),
            "wor": _bf16(Wo[sl, :]),
            "cosP": cosP, "sinP": sinP,
            "maskut": _bf16(maskut), "smat": _bf16(smat),
        })

    nc = _get_nc()
    if _trace:
        _trace = _install_trace_hooks()
    res = run_bass_kernel_spmd(nc, in_maps, core_ids=list(range(NCORES)), trace=_trace)
    _cache["last_res"] = res

    out = np.empty((B, T, C), np.float32)
    for c in range(NCORES):
        b, g = c // 4, c % 4
        out[b][0:3 * 512, g * CPC:(g + 1) * CPC] = res.results[c]["out"].T[0:3 * 512]
        out[b][3 * 512:, g * CPC:(g + 1) * CPC] = \
            np.asarray(res.results[c]["out3"], np.float32).T
    return out
